# revision 16
# baseline (speedup 1.0000x reference)
"""Trainium2 Bass kernel for single-head attention.

reference:
  q = x @ Wq.T ; k = x @ Wk.T ; v = x @ Wv.T        (x: [B,S,D], W*: [D,D])
  out = softmax(q @ k.T / sqrt(D)) @ v              (B=4, S=4096, D=256)

Sharding: 8 cores = (batch b in 0..3) x (query-half h in 0..1).
Each core receives x^T for its batch, columns permuted so its 2048 queries
are columns 0:2048 (attention is permutation-invariant over keys, so K/V
built from the permuted sequence give identical results).  Host passes
transposed inputs (x^T, Wq^T, Wk^T, Wv^T) so the device does no layout
transposes.

Each core computes (fp32r matmuls):
  K^T [256,4096], Q^T [256,2048], V [4096,256]
then a flash-style pass over 128-key chunks:
  S^T = K_chunk @ Q^T  -> exp(S^T/16) = P^T (ACT; no max subtraction: scores
  are ~N(0,1) so exp cannot overflow in fp32)
  O^T += V_chunk.T @ P^T  (PE) ;  pacc += P^T  (DVE, elementwise)
  sums = ones.T @ pacc (replicated on all rows) ; out = O^T * (1/sums)
Core output is O^T [256, 2048]; the host transposes and scatters.
"""

from contextlib import ExitStack

import numpy as np

B, S, D = 4, 4096, 256
H = S // 2          # queries per core
NCORE = 8
KC = S // 128       # 32 key chunks
QT = H // 512       # 4 query tiles
SCALE = 1.0 / np.sqrt(D)

_compiled_nc = None


def _build():
    import concourse.mybir as mybir
    import concourse.tile as tile
    from concourse import bacc

    F32 = mybir.dt.float32
    FR = mybir.dt.float32r
    EXP = mybir.ActivationFunctionType.Exp

    nc = bacc.Bacc("TRN2", target_bir_lowering=False, debug=False, num_devices=NCORE)
    xt = nc.dram_tensor("xt", [D, S], F32, kind="ExternalInput")
    wqt_d = nc.dram_tensor("wqt", [D, D], F32, kind="ExternalInput")
    wkt_d = nc.dram_tensor("wkt", [D, D], F32, kind="ExternalInput")
    wvt_d = nc.dram_tensor("wvt", [D, D], F32, kind="ExternalInput")
    ot = nc.dram_tensor("ot", [D, H], F32, kind="ExternalOutput")

    with tile.TileContext(nc) as tc, ExitStack() as ctx:
        const = ctx.enter_context(tc.tile_pool(name="const", bufs=1))
        big = ctx.enter_context(tc.tile_pool(name="big", bufs=1))
        pt_pool = ctx.enter_context(tc.tile_pool(name="ptp", bufs=6))
        small = ctx.enter_context(tc.tile_pool(name="small", bufs=2))

        _cp_flip = [0]

        def copy_out(dst, srcap):
            # alternate PSUM->SBUF evacuation between DVE and ACT
            _cp_flip[0] ^= 1
            if _cp_flip[0]:
                nc.vector.tensor_copy(dst, srcap)
            else:
                nc.scalar.copy(dst, srcap)

        ones_f = const.tile([128, 128], F32, name="ones_f")
        nc.vector.memset(ones_f, 1.0)
        ones_r = const.tile([128, 128], FR, name="ones_r")
        nc.vector.tensor_copy(ones_r, ones_f)

        # pre-transposed weights: w*t [128, dc, a] = W.T[dc*128 + p, a]
        wqt = const.tile([128, 2, 256], FR, name="wqt")
        wkt = const.tile([128, 2, 256], FR, name="wkt")
        wvt = const.tile([128, 2, 256], FR, name="wvt")
        for dst, src in ((wkt, wkt_d), (wqt, wqt_d), (wvt, wvt_d)):
            nc.gpsimd.dma_start(dst, src[:, :].rearrange("(c p) a -> p c a", p=128).bitcast(FR))

        # persistent tensors
        xT = big.tile([128, 2, KC, 128], FR, name="xT")
        kT = [big.tile([128, KC, 128], FR, name=f"kT{ac}") for ac in range(2)]
        qT = [big.tile([128, QT, 512], FR, name=f"qT{ac}") for ac in range(2)]
        vt = big.tile([128, KC, 256], FR, name="vt")
        osb = [big.tile([128, QT, 512], F32, name=f"osb{ec}") for ec in range(2)]

        # x^T load: [256, 4096] -> [128 part, 2 dc, 32 block, 128], chunked DMAs
        # (smaller leading chunks so the first projections can start earlier)
        xt_r = xt[:, :].rearrange("(c p) (n f) -> p c n f", p=128, f=128).bitcast(FR)
        edges = [0, 4, 8, 16, 24, 32]
        for c in range(len(edges) - 1):
            sl = slice(edges[c], edges[c + 1])
            nc.sync.dma_start(xT[:, :, sl, :], xt_r[:, :, sl, :])

        # ---- phase 1: project K/Q/V, chunk-pipelined with the x^T DMAs ----
        with ExitStack() as p1:
            pj_pool = p1.enter_context(tc.tile_pool(name="pj_psum", bufs=4, space="PSUM"))
            pv_pool = p1.enter_context(tc.tile_pool(name="pv_psum", bufs=4, space="PSUM"))

            for c in range(4):
                # K^T for this chunk's two 512-wide s tiles
                for ac in range(2):
                    for h in range(2):
                        g2 = c * 2 + h
                        pk = pj_pool.tile([128, 512], F32, tag="pj", name=f"pk{ac}{g2}")
                        nc.tensor.matmul(pk, wkt[:, 0, ac * 128:(ac + 1) * 128], xT[:, 0, g2 * 4:(g2 + 1) * 4, :], start=True, stop=False)
                        nc.tensor.matmul(pk, wkt[:, 1, ac * 128:(ac + 1) * 128], xT[:, 1, g2 * 4:(g2 + 1) * 4, :], start=False, stop=True)
                        copy_out(kT[ac][:, g2 * 4:(g2 + 1) * 4, :], pk)
                # Q^T (first two chunks only: q rows 0..2047)
                if c < 2:
                    for ac in range(2):
                        for h in range(2):
                            j = c * 2 + h
                            pq = pj_pool.tile([128, 512], F32, tag="pj", name=f"pq{ac}{j}")
                            nc.tensor.matmul(pq, wqt[:, 0, ac * 128:(ac + 1) * 128], xT[:, 0, j * 4:(j + 1) * 4, :], start=True, stop=False)
                            nc.tensor.matmul(pq, wqt[:, 1, ac * 128:(ac + 1) * 128], xT[:, 1, j * 4:(j + 1) * 4, :], start=False, stop=True)
                            copy_out(qT[ac][:, j, :], pq)
                # V for this chunk's 8 blocks
                for nb in range(8):
                    n = c * 8 + nb
                    pv = pv_pool.tile([128, 256], F32, tag="pv", name=f"pv{n}")
                    nc.tensor.matmul(pv, xT[:, 0, n, :], wvt[:, 0, :], start=True, stop=False)
                    nc.tensor.matmul(pv, xT[:, 1, n, :], wvt[:, 1, :], start=False, stop=True)
                    copy_out(vt[:, n, :], pv)

        # ---- phase 2: flash attention over key chunks ----
        with ExitStack() as p2:
            st_pool = p2.enter_context(tc.tile_pool(name="st_psum", bufs=2, space="PSUM"))
            acc_pool = p2.enter_context(tc.tile_pool(name="acc_psum", bufs=2, space="PSUM"))

            for j in range(QT):
                ot0 = acc_pool.tile([128, 512], F32, tag="ot0", name=f"ot0_{j}")
                ot1 = acc_pool.tile([128, 512], F32, tag="ot1", name=f"ot1_{j}")
                pacc = small.tile([128, 2, 512], FR, tag="pacc", name=f"pacc{j}")
                for g in range(KC // 2):
                    st = st_pool.tile([128, 2, 512], F32, tag="st", name=f"st{j}_{g}")
                    for u in range(2):
                        kc = g * 2 + u
                        nc.tensor.matmul(st[:, u, :], kT[0][:, kc, :], qT[0][:, j, :], start=True, stop=False)
                        nc.tensor.matmul(st[:, u, :], kT[1][:, kc, :], qT[1][:, j, :], start=False, stop=True)
                    pt = pt_pool.tile([128, 2, 512], FR, tag="pt", name=f"pt{j}_{g}")
                    nc.scalar.activation(pt, st, EXP, scale=float(SCALE))
                    # accumulate exp tiles elementwise on DVE (softmax denominator:
                    # cross-partition sum happens once at the end via ones-matmul)
                    if g == 0:
                        nc.vector.tensor_copy(pacc, pt)
                    else:
                        nc.vector.tensor_add(pacc, pacc, pt)
                    for u in range(2):
                        kc = g * 2 + u
                        first, last = kc == 0, kc == KC - 1
                        nc.tensor.matmul(ot0, vt[:, kc, 0:128], pt[:, u, :], start=first, stop=last)
                        nc.tensor.matmul(ot1, vt[:, kc, 128:256], pt[:, u, :], start=first, stop=last)
                # softmax denominator: borrow an st-pool slot for the sums matmul
                smt = st_pool.tile([128, 2, 512], F32, tag="st", name=f"smt{j}")
                sm = smt[:, 0, :]
                for u in range(2):
                    nc.tensor.matmul(sm, ones_r, pacc[:, u, :], start=(u == 0), stop=(u == 1))
                rc = small.tile([128, 512], F32, tag="rc", name=f"rc{j}")
                nc.vector.reciprocal_approx_fast(rc, sm)
                for ec, acc in ((0, ot0), (1, ot1)):
                    for hh in range(2):
                        sl = slice(hh * 256, (hh + 1) * 256)
                        nc.vector.tensor_mul(osb[ec][:, j, sl], acc[:, sl], rc[:, sl])
                        nc.sync.dma_start(
                            ot[ec * 128:(ec + 1) * 128, j * 512 + hh * 256:j * 512 + (hh + 1) * 256],
                            osb[ec][:, j, sl],
                        )

    nc.compile()
    return nc


def _get_nc():
    global _compiled_nc
    if _compiled_nc is None:
        _compiled_nc = _build()
    return _compiled_nc


def make_in_maps(x, Wq, Wk, Wv):
    x = np.asarray(x, dtype=np.float32)
    wqT = np.ascontiguousarray(np.asarray(Wq, dtype=np.float32).T)
    wkT = np.ascontiguousarray(np.asarray(Wk, dtype=np.float32).T)
    wvT = np.ascontiguousarray(np.asarray(Wv, dtype=np.float32).T)
    in_maps = []
    for c in range(NCORE):
        b, h = c // 2, c % 2
        xb = x[b]
        if h == 1:
            xb = np.concatenate([xb[H:], xb[:H]], axis=0)
        in_maps.append({
            "xt": np.ascontiguousarray(xb.T),
            "wqt": wqT,
            "wkt": wkT,
            "wvt": wvT,
        })
    return in_maps


def kernel(x, Wq, Wk, Wv):
    from concourse.bass_utils import run_bass_kernel_spmd

    nc = _get_nc()
    in_maps = make_in_maps(x, Wq, Wk, Wv)
    res = run_bass_kernel_spmd(nc, in_maps, core_ids=list(range(NCORE)))
    out = np.empty((B, S, D), dtype=np.float32)
    for c in range(NCORE):
        b, h = c // 2, c % 2
        out[b, h * H:(h + 1) * H, :] = res.results[c]["ot"].T
    return out


# revision 17
# speedup vs baseline: 1.0021x; 1.0021x over previous
"""Trainium2 Bass kernel for single-head attention.

reference:
  q = x @ Wq.T ; k = x @ Wk.T ; v = x @ Wv.T        (x: [B,S,D], W*: [D,D])
  out = softmax(q @ k.T / sqrt(D)) @ v              (B=4, S=4096, D=256)

Sharding: 8 cores = (batch b in 0..3) x (query-half h in 0..1).
Each core receives x^T for its batch, columns permuted so its 2048 queries
are columns 0:2048 (attention is permutation-invariant over keys, so K/V
built from the permuted sequence give identical results).  Host passes
transposed inputs (x^T, Wq^T, Wk^T, Wv^T) so the device does no layout
transposes.

Each core computes (fp32r matmuls):
  K^T [256,4096], Q^T [256,2048], V [4096,256]
then a flash-style pass over 128-key chunks:
  S^T = K_chunk @ Q^T  -> exp(S^T/16) = P^T (ACT; no max subtraction: scores
  are ~N(0,1) so exp cannot overflow in fp32)
  O^T += V_chunk.T @ P^T  (PE) ;  pacc += P^T  (DVE, elementwise)
  sums = ones.T @ pacc (replicated on all rows) ; out = O^T * (1/sums)
Core output is O^T [256, 2048]; the host transposes and scatters.
"""

from contextlib import ExitStack

import numpy as np

B, S, D = 4, 4096, 256
H = S // 2          # queries per core
NCORE = 8
KC = S // 128       # 32 key chunks
QT = H // 512       # 4 query tiles
SCALE = 1.0 / np.sqrt(D)

_compiled_nc = None


def _build():
    import concourse.mybir as mybir
    import concourse.tile as tile
    from concourse import bacc

    F32 = mybir.dt.float32
    FR = mybir.dt.float32r
    EXP = mybir.ActivationFunctionType.Exp

    nc = bacc.Bacc("TRN2", target_bir_lowering=False, debug=False, num_devices=NCORE)
    xt = nc.dram_tensor("xt", [D, S], F32, kind="ExternalInput")
    wqt_d = nc.dram_tensor("wqt", [D, D], F32, kind="ExternalInput")
    wkt_d = nc.dram_tensor("wkt", [D, D], F32, kind="ExternalInput")
    wvt_d = nc.dram_tensor("wvt", [D, D], F32, kind="ExternalInput")
    ot = nc.dram_tensor("ot", [D, H], F32, kind="ExternalOutput")

    with tile.TileContext(nc) as tc, ExitStack() as ctx:
        const = ctx.enter_context(tc.tile_pool(name="const", bufs=1))
        big = ctx.enter_context(tc.tile_pool(name="big", bufs=1))
        pt_pool = ctx.enter_context(tc.tile_pool(name="ptp", bufs=6))
        small = ctx.enter_context(tc.tile_pool(name="small", bufs=2))

        _cp_flip = [0]

        def copy_out(dst, srcap):
            # alternate PSUM->SBUF evacuation between DVE and ACT
            _cp_flip[0] ^= 1
            if _cp_flip[0]:
                nc.vector.tensor_copy(dst, srcap)
            else:
                nc.scalar.copy(dst, srcap)

        ones_f = const.tile([128, 128], F32, name="ones_f")
        nc.vector.memset(ones_f, 1.0)
        ones_r = const.tile([128, 128], FR, name="ones_r")
        nc.vector.tensor_copy(ones_r, ones_f)

        # pre-transposed weights: w*t [128, dc, a] = W.T[dc*128 + p, a]
        wqt = const.tile([128, 2, 256], FR, name="wqt")
        wkt = const.tile([128, 2, 256], FR, name="wkt")
        wvt = const.tile([128, 2, 256], FR, name="wvt")
        for dst, src in ((wkt, wkt_d), (wqt, wqt_d), (wvt, wvt_d)):
            nc.gpsimd.dma_start(dst, src[:, :].rearrange("(c p) a -> p c a", p=128).bitcast(FR))

        # persistent tensors
        xT = big.tile([128, 2, KC, 128], FR, name="xT")
        kT = [big.tile([128, KC, 128], FR, name=f"kT{ac}") for ac in range(2)]
        qT = [big.tile([128, QT, 512], FR, name=f"qT{ac}") for ac in range(2)]
        vt = big.tile([128, KC, 256], FR, name="vt")
        osb = [big.tile([128, QT, 512], F32, name=f"osb{ec}") for ec in range(2)]

        # x^T load: [256, 4096] -> [128 part, 2 dc, 32 block, 128], chunked DMAs
        # (smaller leading chunks so the first projections can start earlier)
        xt_r = xt[:, :].rearrange("(c p) (n f) -> p c n f", p=128, f=128).bitcast(FR)
        edges = [0, 4, 8, 16, 24, 32]
        for c in range(len(edges) - 1):
            sl = slice(edges[c], edges[c + 1])
            nc.sync.dma_start(xT[:, :, sl, :], xt_r[:, :, sl, :])

        # ---- phase 1: project K/Q/V, chunk-pipelined with the x^T DMAs ----
        with ExitStack() as p1:
            pj_pool = p1.enter_context(tc.tile_pool(name="pj_psum", bufs=4, space="PSUM"))
            pv_pool = p1.enter_context(tc.tile_pool(name="pv_psum", bufs=4, space="PSUM"))

            for g2 in range(8):
                # K^T for s-tile g2 (blocks g2*4 .. g2*4+3)
                for ac in range(2):
                    pk = pj_pool.tile([128, 512], F32, tag="pj", name=f"pk{ac}{g2}")
                    nc.tensor.matmul(pk, wkt[:, 0, ac * 128:(ac + 1) * 128], xT[:, 0, g2 * 4:(g2 + 1) * 4, :], start=True, stop=False)
                    nc.tensor.matmul(pk, wkt[:, 1, ac * 128:(ac + 1) * 128], xT[:, 1, g2 * 4:(g2 + 1) * 4, :], start=False, stop=True)
                    copy_out(kT[ac][:, g2 * 4:(g2 + 1) * 4, :], pk)
                # Q^T (first half only: q rows 0..2047)
                if g2 < 4:
                    for ac in range(2):
                        pq = pj_pool.tile([128, 512], F32, tag="pj", name=f"pq{ac}{g2}")
                        nc.tensor.matmul(pq, wqt[:, 0, ac * 128:(ac + 1) * 128], xT[:, 0, g2 * 4:(g2 + 1) * 4, :], start=True, stop=False)
                        nc.tensor.matmul(pq, wqt[:, 1, ac * 128:(ac + 1) * 128], xT[:, 1, g2 * 4:(g2 + 1) * 4, :], start=False, stop=True)
                        copy_out(qT[ac][:, g2, :], pq)
                # V for these 4 blocks
                for nb in range(4):
                    n = g2 * 4 + nb
                    pv = pv_pool.tile([128, 256], F32, tag="pv", name=f"pv{n}")
                    nc.tensor.matmul(pv, xT[:, 0, n, :], wvt[:, 0, :], start=True, stop=False)
                    nc.tensor.matmul(pv, xT[:, 1, n, :], wvt[:, 1, :], start=False, stop=True)
                    copy_out(vt[:, n, :], pv)

        # ---- phase 2: flash attention over key chunks ----
        with ExitStack() as p2:
            st_pool = p2.enter_context(tc.tile_pool(name="st_psum", bufs=2, space="PSUM"))
            acc_pool = p2.enter_context(tc.tile_pool(name="acc_psum", bufs=2, space="PSUM"))

            for j in range(QT):
                ot0 = acc_pool.tile([128, 512], F32, tag="ot0", name=f"ot0_{j}")
                ot1 = acc_pool.tile([128, 512], F32, tag="ot1", name=f"ot1_{j}")
                pacc = small.tile([128, 2, 512], FR, tag="pacc", name=f"pacc{j}")
                for g in range(KC // 2):
                    st = st_pool.tile([128, 2, 512], F32, tag="st", name=f"st{j}_{g}")
                    for u in range(2):
                        kc = g * 2 + u
                        nc.tensor.matmul(st[:, u, :], kT[0][:, kc, :], qT[0][:, j, :], start=True, stop=False)
                        nc.tensor.matmul(st[:, u, :], kT[1][:, kc, :], qT[1][:, j, :], start=False, stop=True)
                    pt = pt_pool.tile([128, 2, 512], FR, tag="pt", name=f"pt{j}_{g}")
                    nc.scalar.activation(pt, st, EXP, scale=float(SCALE))
                    # accumulate exp tiles elementwise on DVE (softmax denominator:
                    # cross-partition sum happens once at the end via ones-matmul)
                    if g == 0:
                        nc.vector.tensor_copy(pacc, pt)
                    else:
                        nc.vector.tensor_add(pacc, pacc, pt)
                    for u in range(2):
                        kc = g * 2 + u
                        first, last = kc == 0, kc == KC - 1
                        nc.tensor.matmul(ot0, vt[:, kc, 0:128], pt[:, u, :], start=first, stop=last)
                        nc.tensor.matmul(ot1, vt[:, kc, 128:256], pt[:, u, :], start=first, stop=last)
                # softmax denominator: borrow an st-pool slot for the sums matmul
                smt = st_pool.tile([128, 2, 512], F32, tag="st", name=f"smt{j}")
                sm = smt[:, 0, :]
                for u in range(2):
                    nc.tensor.matmul(sm, ones_r, pacc[:, u, :], start=(u == 0), stop=(u == 1))
                rc = small.tile([128, 512], F32, tag="rc", name=f"rc{j}")
                nc.vector.reciprocal_approx_fast(rc, sm)
                for ec, acc in ((0, ot0), (1, ot1)):
                    for hh in range(2):
                        sl = slice(hh * 256, (hh + 1) * 256)
                        nc.vector.tensor_mul(osb[ec][:, j, sl], acc[:, sl], rc[:, sl])
                        nc.sync.dma_start(
                            ot[ec * 128:(ec + 1) * 128, j * 512 + hh * 256:j * 512 + (hh + 1) * 256],
                            osb[ec][:, j, sl],
                        )

    nc.compile()
    return nc


def _get_nc():
    global _compiled_nc
    if _compiled_nc is None:
        _compiled_nc = _build()
    return _compiled_nc


def make_in_maps(x, Wq, Wk, Wv):
    x = np.asarray(x, dtype=np.float32)
    wqT = np.ascontiguousarray(np.asarray(Wq, dtype=np.float32).T)
    wkT = np.ascontiguousarray(np.asarray(Wk, dtype=np.float32).T)
    wvT = np.ascontiguousarray(np.asarray(Wv, dtype=np.float32).T)
    in_maps = []
    for c in range(NCORE):
        b, h = c // 2, c % 2
        xb = x[b]
        if h == 1:
            xb = np.concatenate([xb[H:], xb[:H]], axis=0)
        in_maps.append({
            "xt": np.ascontiguousarray(xb.T),
            "wqt": wqT,
            "wkt": wkT,
            "wvt": wvT,
        })
    return in_maps


def kernel(x, Wq, Wk, Wv):
    from concourse.bass_utils import run_bass_kernel_spmd

    nc = _get_nc()
    in_maps = make_in_maps(x, Wq, Wk, Wv)
    res = run_bass_kernel_spmd(nc, in_maps, core_ids=list(range(NCORE)))
    out = np.empty((B, S, D), dtype=np.float32)
    for c in range(NCORE):
        b, h = c // 2, c % 2
        out[b, h * H:(h + 1) * H, :] = res.results[c]["ot"].T
    return out


# revision 18
# speedup vs baseline: 1.0127x; 1.0106x over previous
"""Trainium2 Bass kernel for single-head attention.

reference:
  q = x @ Wq.T ; k = x @ Wk.T ; v = x @ Wv.T        (x: [B,S,D], W*: [D,D])
  out = softmax(q @ k.T / sqrt(D)) @ v              (B=4, S=4096, D=256)

Sharding: 8 cores = (batch b in 0..3) x (query-half h in 0..1).
Each core receives x^T for its batch, columns permuted so its 2048 queries
are columns 0:2048 (attention is permutation-invariant over keys, so K/V
built from the permuted sequence give identical results).  Host passes
transposed inputs (x^T, Wq^T, Wk^T, Wv^T) so the device does no layout
transposes.

Each core computes (fp32r matmuls):
  K^T [256,4096], Q^T [256,2048], V [4096,256]
then a flash-style pass over 128-key chunks:
  S^T = K_chunk @ Q^T  -> exp(S^T/16) = P^T (ACT; no max subtraction: scores
  are ~N(0,1) so exp cannot overflow in fp32)
  O^T += V_chunk.T @ P^T  (PE) ;  pacc += P^T  (DVE, elementwise)
  sums = ones.T @ pacc (replicated on all rows) ; out = O^T * (1/sums)
Core output is O^T [256, 2048]; the host transposes and scatters.
"""

from contextlib import ExitStack

import numpy as np

B, S, D = 4, 4096, 256
H = S // 2          # queries per core
NCORE = 8
KC = S // 128       # 32 key chunks
QT = H // 512       # 4 query tiles
SCALE = 1.0 / np.sqrt(D)

_compiled_nc = None


def _build():
    import concourse.mybir as mybir
    import concourse.tile as tile
    from concourse import bacc

    F32 = mybir.dt.float32
    FR = mybir.dt.float32r
    EXP = mybir.ActivationFunctionType.Exp

    nc = bacc.Bacc("TRN2", target_bir_lowering=False, debug=False, num_devices=NCORE)
    xt = nc.dram_tensor("xt", [D, S], F32, kind="ExternalInput")
    wqt_d = nc.dram_tensor("wqt", [D, D], F32, kind="ExternalInput")
    wkt_d = nc.dram_tensor("wkt", [D, D], F32, kind="ExternalInput")
    wvt_d = nc.dram_tensor("wvt", [D, D], F32, kind="ExternalInput")
    ot = nc.dram_tensor("ot", [D, H], F32, kind="ExternalOutput")

    with tile.TileContext(nc) as tc, ExitStack() as ctx:
        const = ctx.enter_context(tc.tile_pool(name="const", bufs=1))
        big = ctx.enter_context(tc.tile_pool(name="big", bufs=1))
        pt_pool = ctx.enter_context(tc.tile_pool(name="ptp", bufs=6))
        small = ctx.enter_context(tc.tile_pool(name="small", bufs=2))

        _cp_flip = [0]

        def copy_out(dst, srcap):
            # alternate PSUM->SBUF evacuation between DVE and ACT
            _cp_flip[0] ^= 1
            if _cp_flip[0]:
                nc.vector.tensor_copy(dst, srcap)
            else:
                nc.scalar.copy(dst, srcap)

        ones_f = const.tile([128, 128], F32, name="ones_f")
        nc.vector.memset(ones_f, 1.0)
        ones_r = const.tile([128, 128], FR, name="ones_r")
        nc.vector.tensor_copy(ones_r, ones_f)

        # pre-transposed weights: w*t [128, dc, a] = W.T[dc*128 + p, a]
        wqt = const.tile([128, 2, 256], FR, name="wqt")
        wkt = const.tile([128, 2, 256], FR, name="wkt")
        wvt = const.tile([128, 2, 256], FR, name="wvt")
        for dst, src in ((wkt, wkt_d), (wqt, wqt_d), (wvt, wvt_d)):
            nc.gpsimd.dma_start(dst, src[:, :].rearrange("(c p) a -> p c a", p=128).bitcast(FR))

        # persistent tensors
        xT = big.tile([128, 2, KC, 128], FR, name="xT")
        kT = [big.tile([128, KC, 128], FR, name=f"kT{ac}") for ac in range(2)]
        qT = [big.tile([128, QT, 512], FR, name=f"qT{ac}") for ac in range(2)]
        vt = big.tile([128, KC, 256], FR, name="vt")
        osb = [big.tile([128, QT, 512], F32, name=f"osb{ec}") for ec in range(2)]

        # x^T load: [256, 4096] -> [128 part, 2 dc, 32 block, 128], chunked DMAs
        # (smaller leading chunks so the first projections can start earlier)
        xt_r = xt[:, :].rearrange("(c p) (n f) -> p c n f", p=128, f=128).bitcast(FR)
        edges = [0, 4, 8, 16, 24, 32]
        for c in range(len(edges) - 1):
            sl = slice(edges[c], edges[c + 1])
            nc.sync.dma_start(xT[:, :, sl, :], xt_r[:, :, sl, :])

        # ---- phase 1: project K/Q/V, chunk-pipelined with the x^T DMAs ----
        with ExitStack() as p1:
            pj_pool = p1.enter_context(tc.tile_pool(name="pj_psum", bufs=4, space="PSUM"))
            pv_pool = p1.enter_context(tc.tile_pool(name="pv_psum", bufs=4, space="PSUM"))

            for g2 in range(8):
                # K^T for s-tile g2 (blocks g2*4 .. g2*4+3)
                for ac in range(2):
                    pk = pj_pool.tile([128, 512], F32, tag="pj", name=f"pk{ac}{g2}")
                    nc.tensor.matmul(pk, wkt[:, 0, ac * 128:(ac + 1) * 128], xT[:, 0, g2 * 4:(g2 + 1) * 4, :], start=True, stop=False)
                    nc.tensor.matmul(pk, wkt[:, 1, ac * 128:(ac + 1) * 128], xT[:, 1, g2 * 4:(g2 + 1) * 4, :], start=False, stop=True)
                    copy_out(kT[ac][:, g2 * 4:(g2 + 1) * 4, :], pk)
                # Q^T (first half only: q rows 0..2047)
                if g2 < 4:
                    for ac in range(2):
                        pq = pj_pool.tile([128, 512], F32, tag="pj", name=f"pq{ac}{g2}")
                        nc.tensor.matmul(pq, wqt[:, 0, ac * 128:(ac + 1) * 128], xT[:, 0, g2 * 4:(g2 + 1) * 4, :], start=True, stop=False)
                        nc.tensor.matmul(pq, wqt[:, 1, ac * 128:(ac + 1) * 128], xT[:, 1, g2 * 4:(g2 + 1) * 4, :], start=False, stop=True)
                        copy_out(qT[ac][:, g2, :], pq)
                # V for these 4 blocks
                for nb in range(4):
                    n = g2 * 4 + nb
                    pv = pv_pool.tile([128, 256], F32, tag="pv", name=f"pv{n}")
                    nc.tensor.matmul(pv, xT[:, 0, n, :], wvt[:, 0, :], start=True, stop=False)
                    nc.tensor.matmul(pv, xT[:, 1, n, :], wvt[:, 1, :], start=False, stop=True)
                    copy_out(vt[:, n, :], pv)

        # ---- phase 2: flash attention over key chunks ----
        with ExitStack() as p2:
            st_pool = p2.enter_context(tc.tile_pool(name="st_psum", bufs=2, space="PSUM"))
            acc_pool = p2.enter_context(tc.tile_pool(name="acc_psum", bufs=1, space="PSUM"))

            for j in range(QT):
                ot0 = acc_pool.tile([128, 512], F32, tag="ot0", name=f"ot0_{j}")
                ot1 = acc_pool.tile([128, 512], F32, tag="ot1", name=f"ot1_{j}")
                pacc = small.tile([128, 2, 512], FR, tag="pacc", name=f"pacc{j}")
                for g in range(KC // 2):
                    st = st_pool.tile([128, 2, 512], F32, tag="st", name=f"st{j}_{g}")
                    for u in range(2):
                        kc = g * 2 + u
                        nc.tensor.matmul(st[:, u, :], kT[0][:, kc, :], qT[0][:, j, :], start=True, stop=False)
                        nc.tensor.matmul(st[:, u, :], kT[1][:, kc, :], qT[1][:, j, :], start=False, stop=True)
                    pt = pt_pool.tile([128, 2, 512], FR, tag="pt", name=f"pt{j}_{g}")
                    nc.scalar.activation(pt, st, EXP, scale=float(SCALE))
                    # accumulate exp tiles elementwise on DVE (softmax denominator:
                    # cross-partition sum happens once at the end via ones-matmul)
                    if g == 0:
                        nc.vector.tensor_copy(pacc, pt)
                    else:
                        nc.vector.tensor_add(pacc, pacc, pt)
                    for u in range(2):
                        kc = g * 2 + u
                        first, last = kc == 0, kc == KC - 1
                        nc.tensor.matmul(ot0, vt[:, kc, 0:128], pt[:, u, :], start=first, stop=last)
                        nc.tensor.matmul(ot1, vt[:, kc, 128:256], pt[:, u, :], start=first, stop=last)
                # softmax denominator
                smt = acc_pool.tile([128, 512], F32, tag="sm", name=f"smt{j}")
                sm = smt[:, :]
                for u in range(2):
                    nc.tensor.matmul(sm, ones_r, pacc[:, u, :], start=(u == 0), stop=(u == 1))
                rc = small.tile([128, 512], F32, tag="rc", name=f"rc{j}")
                nc.vector.reciprocal_approx_fast(rc, sm)
                for ec, acc in ((0, ot0), (1, ot1)):
                    for hh in range(2):
                        sl = slice(hh * 256, (hh + 1) * 256)
                        nc.vector.tensor_mul(osb[ec][:, j, sl], acc[:, sl], rc[:, sl])
                        nc.sync.dma_start(
                            ot[ec * 128:(ec + 1) * 128, j * 512 + hh * 256:j * 512 + (hh + 1) * 256],
                            osb[ec][:, j, sl],
                        )

    nc.compile()
    return nc


def _get_nc():
    global _compiled_nc
    if _compiled_nc is None:
        _compiled_nc = _build()
    return _compiled_nc


def make_in_maps(x, Wq, Wk, Wv):
    x = np.asarray(x, dtype=np.float32)
    wqT = np.ascontiguousarray(np.asarray(Wq, dtype=np.float32).T)
    wkT = np.ascontiguousarray(np.asarray(Wk, dtype=np.float32).T)
    wvT = np.ascontiguousarray(np.asarray(Wv, dtype=np.float32).T)
    in_maps = []
    for c in range(NCORE):
        b, h = c // 2, c % 2
        xb = x[b]
        if h == 1:
            xb = np.concatenate([xb[H:], xb[:H]], axis=0)
        in_maps.append({
            "xt": np.ascontiguousarray(xb.T),
            "wqt": wqT,
            "wkt": wkT,
            "wvt": wvT,
        })
    return in_maps


def kernel(x, Wq, Wk, Wv):
    from concourse.bass_utils import run_bass_kernel_spmd

    nc = _get_nc()
    in_maps = make_in_maps(x, Wq, Wk, Wv)
    res = run_bass_kernel_spmd(nc, in_maps, core_ids=list(range(NCORE)))
    out = np.empty((B, S, D), dtype=np.float32)
    for c in range(NCORE):
        b, h = c // 2, c % 2
        out[b, h * H:(h + 1) * H, :] = res.results[c]["ot"].T
    return out
